# revision 5
# baseline (speedup 1.0000x reference)
"""TRN2 Bass kernel for 2-layer GAT + grouped softmax (nn_Actor_1881195675935).

8-core SPMD. Nodes sharded contiguously (12500/core, padded to 12544 = 98
tiles of 128); edges live with the owner of their dst node in an ELLPACK
layout (partition = dst node, free axis = edge slots, per-core relabeling
minimizes slot padding). Per layer: each core computes [h | a_src] rows for
its nodes (att vectors folded into the weights on host), AllGathers the
table, and fetches per-edge rows with dma_gather (4 SWDGE queues, 256B rows,
int16 indices into 4 subtables). Attention softmax runs on DVE/ACT over the
slot axis (a_dst is a per-partition scalar; slot pads point at a poisoned
row with a_src=-1e4 so exp()==0). The final `index`-grouped softmax uses
baked fp8 one-hot matmuls + a 64KB AllReduce.
"""

import sys

sys.path.insert(0, "/opt/trn_rl_repo")

import numpy as np
import ml_dtypes  # noqa: F401

N = 100000
NPC_REAL = 12500
NPC = 12544               # = 98 * 128
NT = 98
NC = 8
NSUB = 4
SUBROWS = 2 * NPC         # 25088 rows per subtable (< 2**15)
F_IN = 128
H1, C1 = 2, 16
OUT = 16
WSEG = 256
TROW = 64                 # table row f32 elements (256B)
PAD_AS = -1.0e4
PAD_IDX = 12543           # in-subtable row used by padding slots


def _preprocess(x, edge_index, index, W1, att_src1, att_dst1, b1,
                W2, att_src2, att_dst2, b2):
    f32 = np.float32
    src = np.asarray(edge_index[0], dtype=np.int64)
    dst = np.asarray(edge_index[1], dtype=np.int64)
    loops = np.arange(N, dtype=np.int64)
    src = np.concatenate([src, loops]).astype(np.int64)
    dst = np.concatenate([dst, loops]).astype(np.int64)

    owner_dst = dst // NPC_REAL
    qsrc = (src // NPC_REAL) // 2

    ldst = dst - owner_dst * NPC_REAL
    counts = np.zeros((NC, NPC, NSUB), dtype=np.int64)
    np.add.at(counts, (owner_dst, ldst, qsrc), 1)

    orders = np.zeros((NC, NPC), dtype=np.int64)
    for c in range(NC):
        n = counts[c]
        orders[c] = np.lexsort((-n[:, 3], -n[:, 2], -n[:, 1], -n[:, 0]))
    inv_orders = np.argsort(orders, axis=1)

    pos = np.zeros(N, dtype=np.int64)
    for c in range(NC):
        ol = np.arange(NPC_REAL)
        pos[c * NPC_REAL + ol] = c * NPC + inv_orders[c][ol]

    ncounts = np.zeros((NC, NPC, NSUB), dtype=np.int64)
    for c in range(NC):
        ncounts[c] = counts[c][orders[c]]
    S = ncounts.reshape(NC, NT, 128, NSUB).max(axis=(0, 2))   # [NT, NSUB]

    # edges sorted by (core, relabeled dst local, quarter)
    new_ldst = inv_orders[owner_dst, ldst]
    ekey = (owner_dst * NPC + new_ldst) * NSUB + qsrc
    eorder = np.argsort(ekey, kind="stable")
    s_srcsub = (pos[src[eorder]] % SUBROWS).astype(np.int64)
    run_starts = np.zeros(NC * NPC * NSUB + 1, dtype=np.int64)
    np.cumsum(np.bincount(ekey, minlength=NC * NPC * NSUB), out=run_starts[1:])

    widths = S.sum(axis=1)
    idxw_t = 8 * widths
    idxw_off = np.concatenate([[0], np.cumsum(idxw_t)]).astype(np.int64)
    IDXW = int(idxw_off[-1])

    gidx = np.zeros((NC, 128, IDXW), dtype=np.int16)
    for c in range(NC):
        for t in range(NT):
            wt = int(widths[t])
            flat = np.full((wt, 128), PAD_IDX, dtype=np.int64)
            col = 0
            for q in range(NSUB):
                sq = int(S[t, q])
                if sq == 0:
                    continue
                ls = (c * NPC + t * 128 + np.arange(128)) * NSUB + q
                r0 = run_starts[ls]                      # [128]
                nq = ncounts[c, t * 128:(t + 1) * 128, q]  # [128]
                i = np.arange(sq)[:, None]               # [sq, 1]
                mask = i < nq[None, :]
                gidxs = np.minimum(r0[None, :] + i, len(s_srcsub) - 1)
                flat[col:col + sq] = np.where(mask, s_srcsub[gidxs], PAD_IDX)
                col += sq
            w16 = flat.reshape(-1, 16).T.astype(np.int16)   # [16, 8*wt]
            gidx[c, :, idxw_off[t]:idxw_off[t + 1]] = np.tile(w16, (8, 1))

    W1 = np.asarray(W1, f32); W2 = np.asarray(W2, f32)
    as1 = np.asarray(att_src1, f32); ad1 = np.asarray(att_dst1, f32)
    as2 = np.asarray(att_src2, f32); ad2 = np.asarray(att_dst2, f32)
    vs1 = np.stack([W1[:, h * C1:(h + 1) * C1] @ as1[h] for h in range(H1)], 1)
    vd1 = np.stack([W1[:, h * C1:(h + 1) * C1] @ ad1[h] for h in range(H1)], 1)
    wcat1 = np.concatenate([W1, vs1, vd1], axis=1).astype(f32)
    vs2 = (W2 @ as2[0])[:, None]
    vd2 = (W2 @ ad2[0])[:, None]
    wcat2 = np.concatenate([W2, vs2, vd2], axis=1).astype(f32)

    x = np.asarray(x, f32)
    xT = np.zeros((NC, F_IN, NPC), dtype=f32)
    glb = np.zeros((NC, NPC), dtype=np.int64)
    real = np.zeros((NC, NPC), dtype=bool)
    for c in range(NC):
        ol = orders[c]
        is_real = ol < NPC_REAL
        g = np.where(is_real, c * NPC_REAL + np.minimum(ol, NPC_REAL - 1), 0)
        xT[c] = np.where(is_real[:, None], x[g], 0.0).astype(f32).T
        glb[c] = g
        real[c] = is_real

    index = np.asarray(index, np.int64)
    seg = np.zeros((NC, NPC), dtype=np.int64)
    g0 = np.zeros(NC, dtype=np.int64)
    for c in range(NC):
        seg[c] = np.where(real[c], index[glb[c]], 0)
        s = seg[c][real[c]]
        g0[c] = s.min()
        assert s.max() - s.min() < WSEG, "segment window exceeds WSEG"
    f8 = ml_dtypes.float8_e4m3
    # ohf[c]: [NT*128, 256]  (lhsT chunks along free); oht[c]: [NT*128, 256]
    ohf = np.zeros((NC, NT * 128, WSEG), dtype=f8)
    oht = np.zeros((NC, NT * 128, WSEG), dtype=f8)
    for c in range(NC):
        for t in range(NT):
            sl = seg[c, t * 128:(t + 1) * 128] - g0[c]
            m = real[c, t * 128:(t + 1) * 128]
            oh = np.zeros((128, WSEG), dtype=np.float32)
            oh[np.arange(128)[m], sl[m]] = 1.0
            ohf[c, t * 128:(t + 1) * 128] = oh.astype(f8)
            # bwd lhsT chunk k: [128 segs, 128 nodes] -> store as [128, 2*128]
            ohtk = np.concatenate([oh[:, :128].T, oh[:, 128:].T], axis=1)
            oht[c, t * 128:(t + 1) * 128] = ohtk.astype(f8)

    padfix = np.zeros((128, 3), dtype=f32)
    padfix[84:128, :] = PAD_AS

    sidx = np.zeros((NC, 128, 2), dtype=np.int32)
    for c in range(NC):
        for k in range(2):
            sidx[c, :, k] = g0[c] + k * 128 + np.arange(128)

    b1t = np.tile(np.asarray(b1, f32)[None, :], (128, 1)).astype(f32)
    b2t = np.tile(np.asarray(b2, f32)[None, :], (128, 1)).astype(f32)

    per_core = [{
        "xT": np.ascontiguousarray(xT[c]),
        "wcat1": wcat1, "wcat2": wcat2, "b1t": b1t, "b2t": b2t,
        "gidx": np.ascontiguousarray(gidx[c]),
        "padfix": padfix,
        "ohf": np.ascontiguousarray(ohf[c]),
        "oht": np.ascontiguousarray(oht[c]),
        "sidx": np.ascontiguousarray(sidx[c]),
    } for c in range(NC)]
    shared = {"S": S, "idxw_off": idxw_off, "IDXW": IDXW}
    asm = {"glb": glb, "real": real}
    return shared, per_core, asm


def _build(shared):
    import concourse.bass as bass
    import concourse.bacc as bacc
    import concourse.tile as tile
    from concourse import mybir, library_config
    from concourse.masks import make_identity

    S = shared["S"]; idxw_off = shared["idxw_off"]; IDXW = shared["IDXW"]
    f32 = mybir.dt.float32
    bf16 = mybir.dt.bfloat16
    f8 = mybir.dt.float8e4
    i16 = mybir.dt.int16
    AL = mybir.AluOpType
    EXP = mybir.ActivationFunctionType.Exp
    IOA = bass.IndirectOffsetOnAxis

    nc = bacc.Bacc("TRN2", target_bir_lowering=False, debug=False,
                   num_devices=NC, num_swdge_queues=4)

    xT_ext = nc.dram_tensor("xT", [F_IN, NPC], f32, kind="ExternalInput")
    wcat1_ext = nc.dram_tensor("wcat1", [F_IN, 36], f32, kind="ExternalInput")
    wcat2_ext = nc.dram_tensor("wcat2", [32, 18], f32, kind="ExternalInput")
    b1_ext = nc.dram_tensor("b1t", [128, 32], f32, kind="ExternalInput")
    b2_ext = nc.dram_tensor("b2t", [128, 16], f32, kind="ExternalInput")
    gidx_ext = nc.dram_tensor("gidx", [128, IDXW], i16, kind="ExternalInput")
    ohf_ext = nc.dram_tensor("ohf", [NT * 128, WSEG], f8, kind="ExternalInput")
    oht_ext = nc.dram_tensor("oht", [NT * 128, WSEG], f8, kind="ExternalInput")
    sidx_ext = nc.dram_tensor("sidx", [128, 2], mybir.dt.int32, kind="ExternalInput")
    padfix_ext = nc.dram_tensor("padfix", [128, 3], f32, kind="ExternalInput")
    out_ext = nc.dram_tensor("out", [NPC, OUT], f32, kind="ExternalOutput")

    with tile.TileContext(nc) as tc:
        with (
            tc.tile_pool(name="dram", bufs=1, space="DRAM") as dr,
            tc.tile_pool(name="const", bufs=1) as cpool,
            tc.tile_pool(name="sbuf", bufs=3) as sb,
            tc.tile_pool(name="gat", bufs=3) as gp,
            tc.tile_pool(name="psum", bufs=2, space="PSUM") as pp,
            tc.tile_pool(name="psum_seg", bufs=1, space="PSUM") as pseg,
            tc.tile_pool(name="res", bufs=1) as rp,
        ):
            tab1_loc = dr.tile([NPC, TROW], f32, name="tab1_loc")
            tab2_loc = dr.tile([NPC, TROW], f32, name="tab2_loc")
            tab1_full = dr.tile([NC * NPC, TROW], f32, name="tab1_full",
                                addr_space="Shared")
            tab2_full = dr.tile([NC * NPC, TROW], f32, name="tab2_full",
                                addr_space="Shared")
            s_loc = dr.tile([1280, OUT], f32, name="s_loc")
            s_red = dr.tile([1280, OUT], f32, name="s_red", addr_space="Shared")

            nc.gpsimd.load_library(library_config.mlp)

            ident = cpool.tile([128, 128], f32, name="ident")
            make_identity(nc, ident[:])
            wc1 = cpool.tile([F_IN, 36], f32, name="wc1")
            nc.sync.dma_start(out=wc1[:], in_=wcat1_ext[:, :])
            wc2 = cpool.tile([32, 18], f32, name="wc2")
            nc.sync.dma_start(out=wc2[:], in_=wcat2_ext[:, :])
            b1s = cpool.tile([128, 32], f32, name="b1s")
            nc.sync.dma_start(out=b1s[:], in_=b1_ext[:, :])
            b2s = cpool.tile([128, 16], f32, name="b2s")
            nc.sync.dma_start(out=b2s[:], in_=b2_ext[:, :])
            pfx = cpool.tile([128, 3], f32, name="pfx")
            nc.sync.dma_start(out=pfx[:], in_=padfix_ext[:, :])

            ad1_all = rp.tile([128, NT * 2], f32, name="ad1_all")
            ad2_all = rp.tile([128, NT], f32, name="ad2_all")
            e_all = rp.tile([128, NT * OUT], f32, name="e_all")
            ebf_all = rp.tile([128, NT * OUT], bf16, name="ebf_all")
            x2_all = rp.tile([128, NT * 32], f32, name="x2_all")

            # ---- phase 0: table1 rows -------------------------------------
            for t in range(NT):
                xt = sb.tile([128, 128], f32, name=f"xt{t}", tag="xt")
                nc.sync.dma_start(out=xt[:], in_=xT_ext[:, t * 128:(t + 1) * 128])
                hp = pp.tile([128, 36], f32, name=f"hp{t}", tag="hp")
                nc.tensor.matmul(out=hp[:], lhsT=xt[:], rhs=wc1[:],
                                 start=True, stop=True)
                hs = sb.tile([128, 36], f32, name=f"hs{t}", tag="hs")
                nc.vector.tensor_copy(out=hs[:], in_=hp[:])
                if t == NT - 1:
                    nc.vector.tensor_tensor(out=hs[:, 32:34], in0=hs[:, 32:34],
                                            in1=pfx[:, 0:2], op=AL.add)
                nc.vector.tensor_copy(out=ad1_all[:, 2 * t:2 * t + 2],
                                      in_=hs[:, 34:36])
                nc.sync.dma_start(out=tab1_loc[t * 128:(t + 1) * 128, 0:36],
                                  in_=hs[:])

            nc.gpsimd.collective_compute(
                "AllGather", AL.bypass, replica_groups=[list(range(NC))],
                ins=[tab1_loc.opt()], outs=[tab1_full.opt()])

            # ---- phase 1: layer-1 aggregation -----------------------------
            for t in range(NT):
                wt = int(S[t].sum())
                gx = gp.tile([128, wt, TROW], f32, name=f"g1_{t}", tag="g1")
                gi = sb.tile([128, 8 * wt], i16, name=f"gi1_{t}", tag="gi1")
                nc.sync.dma_start(out=gi[:],
                                  in_=gidx_ext[:, idxw_off[t]:idxw_off[t + 1]])
                col = 0
                for q in range(NSUB):
                    sq = int(S[t, q])
                    if sq == 0:
                        continue
                    nidx = 128 * sq
                    nc.gpsimd.dma_gather(
                        gx[:, col:col + sq, :],
                        tab1_full[q * SUBROWS:(q + 1) * SUBROWS, :],
                        gi[:, 8 * col:8 * (col + sq)],
                        nidx, nidx, TROW, queue_num=q, single_packet=False)
                    col += sq
                x2 = sb.tile([128, 32], f32, name=f"x2_{t}", tag="x2")
                for h in range(H1):
                    z = sb.tile([128, wt], f32, name=f"z{t}h{h}", tag=f"z{h}")
                    a_s = gx[:, :, 32 + h:33 + h].rearrange("p w e -> p (w e)")
                    nc.vector.tensor_scalar_add(
                        out=z[:], in0=a_s,
                        scalar1=ad1_all[:, 2 * t + h:2 * t + h + 1])
                    zs = sb.tile([128, wt], f32, name=f"zs{t}h{h}", tag=f"zs{h}")
                    nc.vector.tensor_scalar_mul(out=zs[:], in0=z[:], scalar1=0.2)
                    nc.vector.tensor_tensor(out=z[:], in0=z[:], in1=zs[:],
                                            op=AL.max)
                    ex = sb.tile([128, wt], f32, name=f"ex{t}h{h}", tag=f"ex{h}")
                    nc.scalar.activation(out=ex[:], in_=z[:], func=EXP)
                    msg = sb.tile([128, wt, C1], f32, name=f"msg{t}h{h}", tag="msg")
                    exb = ex[:, :, None].to_broadcast([128, wt, C1])
                    nc.vector.tensor_tensor(out=msg[:], in0=gx[:, :, 16 * h:16 * h + 16],
                                            in1=exb, op=AL.mult)
                    num = sb.tile([128, C1], f32, name=f"num{t}h{h}", tag="num")
                    nc.vector.reduce_sum(out=num[:],
                                         in_=msg[:].rearrange("p w e -> p e w"),
                                         axis=mybir.AxisListType.X)
                    den = sb.tile([128, 1], f32, name=f"den{t}h{h}", tag="den")
                    nc.vector.reduce_sum(out=den[:], in_=ex[:],
                                         axis=mybir.AxisListType.X)
                    nc.vector.tensor_scalar_max(out=den[:], in0=den[:], scalar1=1e-30)
                    rcp = sb.tile([128, 1], f32, name=f"rcp{t}h{h}", tag="rcp")
                    nc.vector.reciprocal(out=rcp[:], in_=den[:])
                    nc.vector.tensor_scalar_mul(out=x2[:, 16 * h:16 * h + 16],
                                                in0=num[:], scalar1=rcp[:, 0:1])
                nc.vector.tensor_tensor(out=x2[:], in0=x2[:], in1=b1s[:], op=AL.add)
                nc.vector.tensor_scalar_max(out=x2[:], in0=x2[:], scalar1=0.0)
                nc.vector.tensor_copy(out=x2_all[:, 32 * t:32 * (t + 1)], in_=x2[:])

            # ---- phase 2: table2 rows -------------------------------------
            for t in range(NT):
                x2tp = pp.tile([32, 128], f32, name=f"x2tp{t}", tag="hp")
                nc.tensor.transpose(out=x2tp[:],
                                    in_=x2_all[:, 32 * t:32 * (t + 1)],
                                    identity=ident[:])
                x2ts = sb.tile([32, 128], f32, name=f"x2ts{t}", tag="x2ts")
                nc.vector.tensor_copy(out=x2ts[:], in_=x2tp[:])
                h2p = pp.tile([128, 18], f32, name=f"h2p{t}", tag="dp")
                nc.tensor.matmul(out=h2p[:], lhsT=x2ts[:], rhs=wc2[:],
                                 start=True, stop=True)
                h2s = sb.tile([128, 18], f32, name=f"h2s{t}", tag="h2s")
                nc.vector.tensor_copy(out=h2s[:], in_=h2p[:])
                if t == NT - 1:
                    nc.vector.tensor_tensor(out=h2s[:, 16:17], in0=h2s[:, 16:17],
                                            in1=pfx[:, 2:3], op=AL.add)
                nc.vector.tensor_copy(out=ad2_all[:, t:t + 1], in_=h2s[:, 17:18])
                nc.sync.dma_start(out=tab2_loc[t * 128:(t + 1) * 128, 0:18],
                                  in_=h2s[:])

            nc.gpsimd.collective_compute(
                "AllGather", AL.bypass, replica_groups=[list(range(NC))],
                ins=[tab2_loc.opt()], outs=[tab2_full.opt()])

            # ---- phase 3: layer-2 aggregation + exp + segment partials ----
            sp = [pseg.tile([128, OUT], f32, name=f"segp{k}") for k in range(2)]
            for t in range(NT):
                wt = int(S[t].sum())
                gx = gp.tile([128, wt, TROW], f32, name=f"g2_{t}", tag="g1")
                gi = sb.tile([128, 8 * wt], i16, name=f"gi2_{t}", tag="gi1")
                nc.sync.dma_start(out=gi[:],
                                  in_=gidx_ext[:, idxw_off[t]:idxw_off[t + 1]])
                col = 0
                for q in range(NSUB):
                    sq = int(S[t, q])
                    if sq == 0:
                        continue
                    nidx = 128 * sq
                    nc.gpsimd.dma_gather(
                        gx[:, col:col + sq, :],
                        tab2_full[q * SUBROWS:(q + 1) * SUBROWS, :],
                        gi[:, 8 * col:8 * (col + sq)],
                        nidx, nidx, TROW, queue_num=q, single_packet=False)
                    col += sq
                z = sb.tile([128, wt], f32, name=f"z2_{t}", tag="z0")
                a_s = gx[:, :, 16:17].rearrange("p w e -> p (w e)")
                nc.vector.tensor_scalar_add(out=z[:], in0=a_s,
                                            scalar1=ad2_all[:, t:t + 1])
                zs = sb.tile([128, wt], f32, name=f"zs2_{t}", tag="zs0")
                nc.vector.tensor_scalar_mul(out=zs[:], in0=z[:], scalar1=0.2)
                nc.vector.tensor_tensor(out=z[:], in0=z[:], in1=zs[:], op=AL.max)
                ex = sb.tile([128, wt], f32, name=f"ex2_{t}", tag="ex0")
                nc.scalar.activation(out=ex[:], in_=z[:], func=EXP)
                msg = sb.tile([128, wt, OUT], f32, name=f"msg2_{t}", tag="msg")
                exb = ex[:, :, None].to_broadcast([128, wt, OUT])
                nc.vector.tensor_tensor(out=msg[:], in0=gx[:, :, 0:16], in1=exb,
                                        op=AL.mult)
                num = sb.tile([128, OUT], f32, name=f"num2_{t}", tag="num")
                nc.vector.reduce_sum(out=num[:],
                                     in_=msg[:].rearrange("p w e -> p e w"),
                                     axis=mybir.AxisListType.X)
                den = sb.tile([128, 1], f32, name=f"den2_{t}", tag="den")
                nc.vector.reduce_sum(out=den[:], in_=ex[:],
                                     axis=mybir.AxisListType.X)
                nc.vector.tensor_scalar_max(out=den[:], in0=den[:], scalar1=1e-30)
                rcp = sb.tile([128, 1], f32, name=f"rcp2_{t}", tag="rcp")
                nc.vector.reciprocal(out=rcp[:], in_=den[:])
                o2 = sb.tile([128, OUT], f32, name=f"o2_{t}", tag="o2")
                nc.vector.tensor_scalar_mul(out=o2[:], in0=num[:],
                                            scalar1=rcp[:, 0:1])
                nc.vector.tensor_tensor(out=o2[:], in0=o2[:], in1=b2s[:], op=AL.add)
                nc.scalar.activation(out=e_all[:, OUT * t:OUT * (t + 1)],
                                     in_=o2[:], func=EXP)
                nc.vector.tensor_copy(out=ebf_all[:, OUT * t:OUT * (t + 1)],
                                      in_=e_all[:, OUT * t:OUT * (t + 1)])
                ohf_t = sb.tile([128, WSEG], f8, name=f"ohf{t}", tag="ohf")
                nc.sync.dma_start(out=ohf_t[:],
                                  in_=ohf_ext[t * 128:(t + 1) * 128, :])
                for k in range(2):
                    nc.tensor.matmul(out=sp[k][:],
                                     lhsT=ohf_t[:, k * 128:(k + 1) * 128],
                                     rhs=ebf_all[:, OUT * t:OUT * (t + 1)],
                                     start=(t == 0), stop=(t == NT - 1))

            # ---- phase 4: combine segment sums across cores ---------------
            zt = sb.tile([128, 160], f32, name="zt")
            nc.vector.memset(zt[:], 0.0)
            nc.sync.dma_start(
                out=s_loc.rearrange("(c p) f -> p c f", p=128),
                in_=zt[:].rearrange("p (c f) -> p c f", c=10))
            sxi = sb.tile([128, 2], mybir.dt.int32, name="sxi")
            nc.sync.dma_start(out=sxi[:], in_=sidx_ext[:, :])
            for k in range(2):
                spc = sb.tile([128, OUT], f32, name=f"spc{k}", tag="spc")
                nc.vector.tensor_copy(out=spc[:], in_=sp[k][:])
                nc.gpsimd.indirect_dma_start(
                    out=s_loc[:, :],
                    out_offset=IOA(ap=sxi[:, k:k + 1], axis=0),
                    in_=spc[:], in_offset=None)

            nc.gpsimd.collective_compute(
                "AllReduce", AL.add, replica_groups=[list(range(NC))],
                ins=[s_loc.opt()], outs=[s_red.opt()])

            sw = []
            for k in range(2):
                swf = sb.tile([128, OUT], f32, name=f"swf{k}", tag="swf")
                nc.gpsimd.indirect_dma_start(
                    out=swf[:], out_offset=None,
                    in_=s_red[:, :],
                    in_offset=IOA(ap=sxi[:, k:k + 1], axis=0))
                swb = rp.tile([128, OUT], bf16, name=f"sw{k}")
                nc.vector.tensor_copy(out=swb[:], in_=swf[:])
                sw.append(swb)

            # ---- phase 5: divide, write out -------------------------------
            for t in range(NT):
                oht_t = sb.tile([128, WSEG], f8, name=f"oht{t}", tag="oht")
                nc.sync.dma_start(out=oht_t[:],
                                  in_=oht_ext[t * 128:(t + 1) * 128, :])
                dp = pp.tile([128, OUT], f32, name=f"dp{t}", tag="dp")
                for k in range(2):
                    nc.tensor.matmul(out=dp[:],
                                     lhsT=oht_t[:, k * 128:(k + 1) * 128],
                                     rhs=sw[k][:], start=(k == 0), stop=(k == 1))
                dd = sb.tile([128, OUT], f32, name=f"dd{t}", tag="dd")
                nc.vector.tensor_scalar_max(out=dd[:], in0=dp[:], scalar1=1e-30)
                nc.vector.reciprocal(out=dd[:], in_=dd[:])
                fo = sb.tile([128, OUT], f32, name=f"fo{t}", tag="fo")
                nc.vector.tensor_tensor(out=fo[:],
                                        in0=e_all[:, OUT * t:OUT * (t + 1)],
                                        in1=dd[:], op=AL.mult)
                nc.sync.dma_start(out=out_ext[t * 128:(t + 1) * 128, :], in_=fo[:])

    nc.compile()
    return nc


def kernel_impl(inputs, trace=False):
    from concourse.bass_utils import run_bass_kernel_spmd
    shared, per_core, asm = _preprocess(**inputs)
    nc = _build(shared)
    res = run_bass_kernel_spmd(nc, per_core, core_ids=list(range(NC)),
                               trace=trace)
    out = np.zeros((N, OUT), dtype=np.float32)
    for c in range(NC):
        o = np.asarray(res.results[c]["out"])
        m = asm["real"][c]
        out[asm["glb"][c][m]] = o[m]
    return out, res


def kernel(**inputs):
    out, _ = kernel_impl(inputs, trace=False)
    return out
